# revision 6
# baseline (speedup 1.0000x reference)
"""Distributed 3-layer GAT encoder on 8 TRN2 NeuronCores (Bass/Tile).

v2 strategy (graph partition by dst):
  - Core c owns dst nodes [2500c, 2500c+2500), padded to 2560 = 20 blocks x 128.
  - Self-loops are NOT in the edge list; their softmax contribution is folded
    into the flush using hloc_sb [P, 20, 264] (local rows [h|as|ad], SBUF
    resident, written by one matmul per block at the previous layer's flush).
  - Per layer, a full node table lives in each core's HBM:
      tab_l [20480, 384|128] fp16 : rows [h | alpha_src | alpha_dst | pad]
    built by matmuls from all-gathered transposed features with folded
    weights [W | W.a_src | W.a_dst].
  - Edge phase: ONE dma_gather per group of 4 dst blocks (the gather has a
    ~18us fixed cost, so fewer+bigger gathers win). Per 128-dst block:
      ind[e,d] by DVE compare of SBUF-resident dloc vs iota (no HBM stream);
      indT from an int8 HBM stream (dlocrep8) vs int8 iota;
      alpha_dst expanded per edge via matmul(lhsT=indT, rhs=hloc ad cols);
      p = exp(leaky_relu(as+ad)) computed on H cols only; the C-broadcast is
      folded into the DVE multiply p*h;
      numerator + denominator accumulated in PSUM via matmuls (lhsT=ind).
  - Flush: add self-loop terms, normalize, mean over heads, bias, relu ->
    PE transpose -> next-layer hloc matmul -> AllGather fp16 -> table rebuild.
"""
import numpy as np

N = 20000
NCORES = 8
NPC = 2500
NPAD = 2560
NBLK = 20
NTOT = NCORES * NPAD  # 20480
P = 128
GB = 4  # dst blocks per merged gather
NGRP = NBLK // GB

LAST_RESULT = None


# ----------------------------------------------------------------- host prep
def _wrap16(idx, ncols):
    n = len(idx)
    w = np.zeros((P, ncols), dtype=np.int16)
    cols = (n + 15) // 16
    assert cols <= ncols
    buf = np.zeros((16, cols), dtype=np.int16)
    buf[np.arange(n) % 16, np.arange(n) // 16] = idx
    for g in range(8):
        w[16 * g:16 * g + 16, :cols] = buf
    return w


def _preprocess(edge_index):
    src = np.asarray(edge_index[0], dtype=np.int64)
    dst = np.asarray(edge_index[1], dtype=np.int64)
    # self-loops handled locally in the flush; NOT added to the edge list

    own_s = src // NPC
    src_p = own_s * NPAD + (src - own_s * NPC)
    own = dst // NPC
    dst_loc = dst - own * NPC

    order = np.lexsort((dst_loc, own))
    src_p, dst_loc, own = src_p[order], dst_loc[order], own[order]
    blk = dst_loc // P
    counts = np.zeros((NCORES, NBLK), dtype=np.int64)
    for c in range(NCORES):
        for b in range(NBLK):
            counts[c, b] = np.sum((own == c) & (blk == b))
    T = np.maximum(1, np.ceil(counts.max(axis=0) / P).astype(np.int64))
    Ttot = int(T.sum())

    wrap_src = np.zeros((NCORES, P, Ttot * 8), dtype=np.int16)
    dstloc16 = np.full((NCORES, P, Ttot), -1.0, dtype=np.float16)
    dlocrep8 = np.full((NCORES, Ttot * P), -1, dtype=np.int8)
    off8 = np.zeros(NBLK + 1, dtype=np.int64)
    offT = np.zeros(NBLK + 1, dtype=np.int64)
    for b in range(NBLK):
        off8[b + 1] = off8[b] + T[b] * 8
        offT[b + 1] = offT[b] + T[b]
    for c in range(NCORES):
        m_c = own == c
        for b in range(NBLK):
            m = m_c & (blk == b)
            cnt = int(counts[c, b])
            nb = int(T[b]) * P
            isrc = np.zeros(nb, dtype=np.int64)
            isrc[:cnt] = src_p[m]
            dl = np.full(nb, -1.0, dtype=np.float32)
            dl[:cnt] = dst_loc[m] - b * P
            wrap_src[c, :, off8[b]:off8[b + 1]] = _wrap16(isrc, int(T[b]) * 8)
            dstloc16[c, :, offT[b]:offT[b + 1]] = (
                dl.reshape(int(T[b]), P).T.astype(np.float16))
            dlocrep8[c, offT[b] * P:offT[b + 1] * P] = dl.astype(np.int8)
    # replicate along partitions: [NCORES, P, Ttot*P]
    dlocrep8 = np.repeat(dlocrep8[:, None, :], P, axis=1)
    return T, off8, offT, wrap_src, dstloc16, dlocrep8


# ------------------------------------------------------------- build program
def _build(T, off8, offT, do_compile=True):
    from concourse import bass, bacc, mybir, tile

    f16 = mybir.dt.float16
    f32 = mybir.dt.float32
    i16 = mybir.dt.int16
    i8 = mybir.dt.int8
    AF = mybir.ActivationFunctionType
    OP = mybir.AluOpType

    Ttot = int(T.sum())
    Tmax = int(T.max())
    NW = Ttot * 8
    NVALID_LAST = NPC - (NBLK - 1) * P  # 68

    nc = bacc.Bacc("TRN2", target_bir_lowering=False, debug=False,
                   num_devices=NCORES)

    # inputs
    xT16 = nc.dram_tensor("xT16", [P, NTOT], f16, kind="ExternalInput")
    xlocT = nc.dram_tensor("xlocT", [P, NPAD], f16, kind="ExternalInput")
    iwsrc = nc.dram_tensor("iwsrc", [P, NW], i16, kind="ExternalInput")
    dloc = nc.dram_tensor("dloc", [P, Ttot], f16, kind="ExternalInput")
    dlocrep8 = nc.dram_tensor("dlocrep8", [P, Ttot * P], i8,
                              kind="ExternalInput")
    iotabig = nc.dram_tensor("iotabig", [P, Tmax * P], f16,
                             kind="ExternalInput")
    iotacrbig8 = nc.dram_tensor("iotacrbig8", [P, Tmax * P], i8,
                                kind="ExternalInput")
    c100 = nc.dram_tensor("c100", [P, 32], f32, kind="ExternalInput")
    c1em8 = nc.dram_tensor("c1em8", [P, 32], f32, kind="ExternalInput")
    ident16 = nc.dram_tensor("ident16", [P, P], f16, kind="ExternalInput")
    identf = nc.dram_tensor("identf", [P, P], f32, kind="ExternalInput")
    # folded weights: [W | W.a_src | W.a_dst]
    w1c = nc.dram_tensor("w1c", [128, 264], f16, kind="ExternalInput")
    w2c = nc.dram_tensor("w2c", [64, 264], f16, kind="ExternalInput")
    w3c = nc.dram_tensor("w3c", [64, 34], f16, kind="ExternalInput")
    b1r = nc.dram_tensor("b1r", [P, 64], f32, kind="ExternalInput")
    b2r = nc.dram_tensor("b2r", [P, 64], f32, kind="ExternalInput")
    b3r = nc.dram_tensor("b3r", [P, 32], f32, kind="ExternalInput")
    bmr = nc.dram_tensor("bmr", [P, 32], f32, kind="ExternalInput")
    bvr = nc.dram_tensor("bvr", [P, 32], f32, kind="ExternalInput")
    wm = nc.dram_tensor("wm", [32, 32], f32, kind="ExternalInput")
    wv = nc.dram_tensor("wv", [32, 32], f32, kind="ExternalInput")

    # outputs
    z_out = nc.dram_tensor("z", [NPC, 32], f32, kind="ExternalOutput")
    zm_out = nc.dram_tensor("zmean", [NPC, 32], f32, kind="ExternalOutput")
    zv_out = nc.dram_tensor("zvar", [NPC, 32], f32, kind="ExternalOutput")

    with tile.TileContext(nc) as tc:
        with (
            tc.tile_pool(name="const", bufs=1) as cpool,
            tc.tile_pool(name="sb", bufs=3) as sb,
            tc.tile_pool(name="gth", bufs=2) as gth,
            tc.tile_pool(name="blk", bufs=2) as blk,
            tc.tile_pool(name="blks", bufs=2) as blks,
            tc.tile_pool(name="psreb", bufs=2, space="PSUM") as psreb,
            tc.tile_pool(name="psad", bufs=1, space="PSUM") as psad,
            tc.tile_pool(name="pssm", bufs=1, space="PSUM") as pssm,
            tc.tile_pool(name="psagg", bufs=2, space="PSUM") as psagg,
            tc.tile_pool(name="dram", bufs=1, space="DRAM") as dram,
        ):
            tab1 = dram.tile([NTOT, 384], f16)
            tab2 = dram.tile([NTOT, 384], f16)
            tab3 = dram.tile([NTOT, 128], f16)
            x2T_loc = dram.tile([64, NPAD], f16)
            x3T_loc = dram.tile([64, NPAD], f16)
            x2T_full = dram.tile([NCORES, 64, NPAD], f16)
            x3T_full = dram.tile([NCORES, 64, NPAD], f16)

            def ld(shape, dt, src):
                t = cpool.tile(shape, dt, tag="c_" + src.name)
                nc.sync.dma_start(out=t[:], in_=src[:, :])
                return t

            id16_sb = ld([P, P], f16, ident16)
            idf_sb = ld([P, P], f32, identf)
            w1c_sb = ld([128, 264], f16, w1c)
            w2c_sb = ld([64, 264], f16, w2c)
            w3c_sb = ld([64, 34], f16, w3c)
            b1r_sb = ld([P, 64], f32, b1r)
            b2r_sb = ld([P, 64], f32, b2r)
            b3r_sb = ld([P, 32], f32, b3r)
            bmr_sb = ld([P, 32], f32, bmr)
            bvr_sb = ld([P, 32], f32, bvr)
            wm_sb = ld([32, 32], f32, wm)
            wv_sb = ld([32, 32], f32, wv)
            iwsrc_sb = ld([P, NW], i16, iwsrc)
            dloc_sb = ld([P, Ttot], f16, dloc)
            iotabig_sb = ld([P, Tmax * P], f16, iotabig)
            iotacrbig8_sb = ld([P, Tmax * P], i8, iotacrbig8)
            c100_sb = ld([P, 32], f32, c100)
            c1em8_sb = ld([P, 32], f32, c1em8)
            xloc_sb = ld([P, NPAD], f16, xlocT)

            # local rows [h | as | ad] of the CURRENT layer, per dst block
            hloc_sb = cpool.tile([P, NBLK, 264], f16, tag="hloc")

            # -------- table rebuild: tab rows = fp16(xT^T @ wc) ----------
            G = 4
            def rebuild(src_getter4, wc_sb, in_c, ncols, tab):
                for t4 in range(NTOT // P // G):
                    e1 = nc.sync if t4 % 2 == 0 else nc.scalar
                    e2 = nc.scalar if t4 % 2 == 0 else nc.sync
                    lh = sb.tile([in_c, G * P], f16, tag="reblh")
                    e1.dma_start(out=lh[:], in_=src_getter4(t4))
                    h16 = sb.tile([P, G, ncols], f16, tag="rebh")
                    for j in range(G):
                        pr = psreb.tile([P, ncols], f32, space="PSUM",
                                        tag="reb")
                        nc.tensor.matmul(
                            out=pr[:], lhsT=lh[:, j * P:(j + 1) * P],
                            rhs=wc_sb[:in_c, :ncols], start=True, stop=True)
                        if j % 2 == 0:
                            nc.vector.tensor_copy(out=h16[:, j, :], in_=pr[:])
                        else:
                            nc.scalar.activation(h16[:, j, :], pr[:], AF.Copy)
                    e2.dma_start(
                        out=tab[t4 * G * P:(t4 + 1) * G * P, 0:ncols]
                        .rearrange("(j r) c -> r j c", j=G),
                        in_=h16[:])

            # -------- hloc for layer 1 (from local transposed x) ---------
            def hloc_init():
                for b in range(NBLK):
                    pr = pssm.tile([P, 264], f32, space="PSUM", tag="hl")
                    nc.tensor.matmul(out=pr[:],
                                     lhsT=xloc_sb[:, b * P:(b + 1) * P],
                                     rhs=w1c_sb[:, :], start=True, stop=True)
                    if b % 2 == 0:
                        nc.vector.tensor_copy(out=hloc_sb[:, b, :], in_=pr[:])
                    else:
                        nc.scalar.activation(hloc_sb[:, b, :], pr[:], AF.Copy)

            # -------- edge phase ------------------------------------------
            def edge_layer(tab, elem, H, C, flush):
                HC = H * C
                for grp in range(NGRP):
                    b0 = grp * GB
                    b1 = b0 + GB
                    Tg = int(offT[b1] - offT[b0])
                    nidx = Tg * P
                    g = gth.tile([P, Tg, elem], f16, tag="g")
                    nc.gpsimd.dma_gather(
                        out_ap=g[:], in_ap=tab[:, :],
                        idxs_ap=iwsrc_sb[:, int(off8[b0]):int(off8[b1])],
                        num_idxs=nidx, num_idxs_reg=nidx, elem_size=elem,
                        elem_step=int(tab.shape[1]),
                        single_packet=nidx <= 1024)
                    for b in range(b0, b1):
                        Tb = int(T[b])
                        to = int(offT[b] - offT[b0])
                        gb = g[:, to:to + Tb, :]
                        dlr8 = blks.tile([P, Tb * P], i8, tag="dlr8")
                        nc.sync.dma_start(
                            out=dlr8[:],
                            in_=dlocrep8[:, int(offT[b]) * P:
                                         int(offT[b + 1]) * P])
                        ind = blks.tile([P, Tb, P], f16, tag="ind")
                        nc.vector.tensor_tensor(
                            out=ind[:],
                            in0=dloc_sb[:, int(offT[b]):int(offT[b + 1]),
                                        None].to_broadcast([P, Tb, P]),
                            in1=iotabig_sb[:, :Tb * P]
                            .rearrange("p (t q) -> p t q", t=Tb),
                            op=OP.is_equal)
                        indT = blks.tile([P, Tb, P], f16, tag="indT")
                        nc.vector.tensor_tensor(
                            out=indT[:].rearrange("p t q -> p (t q)"),
                            in0=iotacrbig8_sb[:, :Tb * P],
                            in1=dlr8[:], op=OP.is_equal)
                        pad_all = psad.tile([P, Tb, H], f32, space="PSUM",
                                            tag="ad")
                        for t in range(Tb):
                            nc.tensor.matmul(
                                out=pad_all[:, t, :],
                                lhsT=indT[:, t, :],
                                rhs=hloc_sb[:, b, HC + H:HC + 2 * H],
                                start=True, stop=True)
                        es = sb.tile([P, Tb, H], f32, tag="es")
                        nc.vector.tensor_add(out=es[:],
                                             in0=gb[:, :, HC:HC + H],
                                             in1=pad_all[:])
                        es2 = sb.tile([P, Tb, H], f32, tag="es2")
                        nc.vector.tensor_scalar_mul(out=es2[:], in0=es[:],
                                                    scalar1=0.2)
                        nc.vector.tensor_max(out=es[:], in0=es[:], in1=es2[:])
                        pex = blk.tile([P, Tb, HC + H], f16, tag="pex")
                        nc.scalar.activation(
                            pex[:, :, 0:HC]
                            .rearrange("p t (h c) -> p t h c", h=H),
                            es[:, :, :, None].to_broadcast([P, Tb, H, C]),
                            AF.Exp)
                        nc.scalar.activation(pex[:, :, HC:HC + H], es[:],
                                             AF.Exp)
                        nc.vector.tensor_mul(out=pex[:, :, 0:HC],
                                             in0=gb[:, :, 0:HC],
                                             in1=pex[:, :, 0:HC])
                        pa = psagg.tile([P, HC + H], f32, space="PSUM",
                                        tag="agg")
                        for t in range(Tb):
                            nc.tensor.matmul(
                                out=pa[:], lhsT=ind[:, t, :],
                                rhs=pex[:, t, :],
                                start=(t == 0), stop=(t == Tb - 1))
                        flush(b, pa)

            # -------- self-loop contribution (p_self, numer, denom) ------
            def self_terms(b, pa, H, C):
                HC = H * C
                est = sb.tile([P, H], f32, tag="est")
                nc.vector.tensor_add(out=est[:],
                                     in0=hloc_sb[:, b, HC:HC + H],
                                     in1=hloc_sb[:, b, HC + H:HC + 2 * H])
                es2t = sb.tile([P, H], f32, tag="es2t")
                nc.vector.tensor_scalar_mul(out=es2t[:], in0=est[:],
                                            scalar1=0.2)
                nc.vector.tensor_max(out=est[:], in0=est[:], in1=es2t[:])
                psf = sb.tile([P, H], f32, tag="psf")
                nc.scalar.activation(psf[:], est[:], AF.Exp)
                den = sb.tile([P, H], f32, tag="den")
                nc.vector.tensor_add(out=den[:], in0=pa[:, HC:HC + H],
                                     in1=psf[:])
                num = sb.tile([P, HC], f32, tag="num")
                nc.vector.tensor_tensor(
                    out=num[:].rearrange("p (h c) -> p h c", h=H),
                    in0=hloc_sb[:, b, 0:HC]
                    .rearrange("p (h c) -> p h c", h=H),
                    in1=psf[:, :, None].to_broadcast([P, H, C]),
                    op=OP.mult)
                nc.vector.tensor_add(out=num[:], in0=num[:], in1=pa[:, 0:HC])
                return num, den

            # -------- flush -----------------------------------------------
            def flush_12(b, pa, H, C, brep_sb, xT_loc_dram, wnext_sb, wn_cols):
                HC = H * C
                num, den = self_terms(b, pa, H, C)
                inv = sb.tile([P, H], f32, tag="inv")
                nc.vector.tensor_scalar_add(out=inv[:], in0=den[:],
                                            scalar1=1e-16)
                nc.vector.reciprocal(out=inv[:], in_=inv[:])
                nc.vector.tensor_scalar_mul(out=inv[:], in0=inv[:],
                                            scalar1=1.0 / H)
                ivx = sb.tile([P, HC], f32, tag="ivx")
                nc.scalar.activation(
                    ivx[:].rearrange("p (h c) -> p h c", h=H),
                    inv[:, :, None].to_broadcast([P, H, C]), AF.Copy)
                nrm = sb.tile([P, HC], f32, tag="nrm")
                nc.vector.tensor_mul(out=nrm[:], in0=num[:], in1=ivx[:])
                m = sb.tile([P, C], f32, tag="mean")
                nc.vector.tensor_reduce(
                    out=m[:], in_=nrm[:].rearrange("p (h c) -> p c h", h=H),
                    axis=mybir.AxisListType.X, op=OP.add)
                nc.vector.tensor_add(out=m[:], in0=m[:], in1=brep_sb[:, :C])
                x16 = sb.tile([P, C], f16, tag="x16")
                nc.scalar.activation(x16[:], m[:], AF.Relu)
                pt = pssm.tile([C, P], f16, space="PSUM", tag="sm")
                nc.tensor.transpose(out=pt[:], in_=x16[:], identity=id16_sb[:])
                xt = sb.tile([C, P], f16, tag="xt")
                nc.scalar.activation(xt[:], pt[:], AF.Copy)
                nc.sync.dma_start(out=xT_loc_dram[:, b * P:(b + 1) * P],
                                  in_=xt[:])
                # next layer's local rows [h | as | ad]
                prh = pssm.tile([P, wn_cols], f32, space="PSUM", tag="hl")
                nc.tensor.matmul(out=prh[:], lhsT=xt[:],
                                 rhs=wnext_sb[:C, :wn_cols],
                                 start=True, stop=True)
                nc.vector.tensor_copy(out=hloc_sb[:, b, 0:wn_cols],
                                      in_=prh[:])

            def flush_3(b, pa):
                nvalid = NVALID_LAST if b == NBLK - 1 else P
                num, den = self_terms(b, pa, 1, 32)
                inv = sb.tile([P, 1], f32, tag="inv")
                nc.vector.tensor_scalar_add(out=inv[:], in0=den[:],
                                            scalar1=1e-16)
                nc.vector.reciprocal(out=inv[:], in_=inv[:])
                z = sb.tile([P, 32], f32, tag="zf")
                nc.vector.tensor_scalar_mul(out=z[:], in0=num[:],
                                            scalar1=inv[:])
                nc.vector.tensor_add(out=z[:], in0=z[:], in1=b3r_sb[:])
                nc.sync.dma_start(out=z_out[b * P:b * P + nvalid, :],
                                  in_=z[:nvalid, :])
                zt_ps = pssm.tile([32, P], f32, space="PSUM", tag="sm")
                nc.tensor.transpose(out=zt_ps[:], in_=z[:, :32],
                                    identity=idf_sb[:])
                zt = sb.tile([32, P], f32, tag="zt")
                nc.vector.tensor_copy(out=zt[:], in_=zt_ps[:])
                pm = pssm.tile([P, 32], f32, space="PSUM", tag="sm2")
                nc.tensor.matmul(out=pm[:], lhsT=zt[:], rhs=wm_sb[:],
                                 start=True, stop=True)
                zm = sb.tile([P, 32], f32, tag="zm")
                nc.vector.tensor_add(out=zm[:], in0=pm[:], in1=bmr_sb[:])
                nc.sync.dma_start(out=zm_out[b * P:b * P + nvalid, :],
                                  in_=zm[:nvalid, :])
                pv = pssm.tile([P, 32], f32, space="PSUM", tag="sm2")
                nc.tensor.matmul(out=pv[:], lhsT=zt[:], rhs=wv_sb[:],
                                 start=True, stop=True)
                zv = sb.tile([P, 32], f32, tag="zv")
                nc.vector.tensor_add(out=zv[:], in0=pv[:], in1=bvr_sb[:])
                nc.scalar.activation(zv[:], zv[:], AF.Exp)
                nc.vector.tensor_tensor(out=zv[:], in0=zv[:], in1=c100_sb[:],
                                        op=OP.min)
                nc.vector.tensor_tensor(out=zv[:], in0=zv[:], in1=c1em8_sb[:],
                                        op=OP.max)
                nc.sync.dma_start(out=zv_out[b * P:b * P + nvalid, :],
                                  in_=zv[:nvalid, :])

            # ================ the program ==================================
            hloc_init()
            rebuild(lambda t4: xT16[:, t4 * G * P:(t4 + 1) * G * P],
                    w1c_sb, 128, 264, tab1)
            edge_layer(tab1, 384, 4, 64,
                       lambda b, pa: flush_12(b, pa, 4, 64, b1r_sb, x2T_loc,
                                              w2c_sb, 264))
            nc.gpsimd.collective_compute(
                "AllGather", mybir.AluOpType.bypass,
                replica_groups=[list(range(NCORES))],
                ins=[x2T_loc[:]], outs=[x2T_full[:]])
            rebuild(lambda t4: x2T_full[(G * t4) // NBLK, :,
                                        ((G * t4) % NBLK) * P:
                                        ((G * t4) % NBLK + G) * P],
                    w2c_sb, 64, 264, tab2)
            edge_layer(tab2, 384, 4, 64,
                       lambda b, pa: flush_12(b, pa, 4, 64, b2r_sb, x3T_loc,
                                              w3c_sb, 34))
            nc.gpsimd.collective_compute(
                "AllGather", mybir.AluOpType.bypass,
                replica_groups=[list(range(NCORES))],
                ins=[x3T_loc[:]], outs=[x3T_full[:]])
            rebuild(lambda t4: x3T_full[(G * t4) // NBLK, :,
                                        ((G * t4) % NBLK) * P:
                                        ((G * t4) % NBLK + G) * P],
                    w3c_sb, 64, 34, tab3)
            edge_layer(tab3, 128, 1, 32, flush_3)

    if do_compile:
        nc.compile()
    return nc


def _make_in_maps(x, params, wrap_src, dstloc16, dlocrep8, Tmax):
    x = np.asarray(x, dtype=np.float32)

    def comb(W, a_s, a_d):
        W = np.asarray(W, np.float32)
        a_s = np.asarray(a_s, np.float32)
        a_d = np.asarray(a_d, np.float32)
        heads, c = a_s.shape
        Wr = W.reshape(W.shape[0], heads, c)
        was = np.einsum('ihc,hc->ih', Wr, a_s)
        wad = np.einsum('ihc,hc->ih', Wr, a_d)
        return np.concatenate([W, was, wad], axis=1).astype(np.float16)

    xT16 = np.zeros((P, NTOT), dtype=np.float16)
    for c in range(NCORES):
        xs = x[c * NPC:(c + 1) * NPC]
        xT16[:, c * NPAD:c * NPAD + NPC] = xs.T.astype(np.float16)

    def rep(v, n=P):
        v = np.asarray(v, np.float32).reshape(1, -1)
        return np.repeat(v, n, axis=0).astype(np.float32)

    iotacr8 = np.tile(np.arange(P, dtype=np.int8).reshape(P, 1),
                      (1, Tmax * P))
    common = dict(
        xT16=xT16,
        iotabig=np.tile(np.arange(P, dtype=np.float16), (P, Tmax)),
        iotacrbig8=iotacr8,
        c100=np.full((P, 32), 100.0, dtype=np.float32),
        c1em8=np.full((P, 32), 1e-8, dtype=np.float32),
        ident16=np.eye(P, dtype=np.float16),
        identf=np.eye(P, dtype=np.float32),
        w1c=comb(params['W1'], params['as1'], params['ad1']),
        w2c=comb(params['W2'], params['as2'], params['ad2']),
        w3c=comb(params['W3'], params['as3'], params['ad3']),
        b1r=rep(params['b1']), b2r=rep(params['b2']), b3r=rep(params['b3']),
        bmr=rep(params['bm']), bvr=rep(params['bv']),
        wm=np.asarray(params['Wm'], np.float32),
        wv=np.asarray(params['Wv'], np.float32),
    )
    in_maps = []
    for c in range(NCORES):
        m = dict(common)
        m.update(iwsrc=wrap_src[c], dloc=dstloc16[c], dlocrep8=dlocrep8[c],
                 xlocT=xT16[:, c * NPAD:(c + 1) * NPAD].copy())
        in_maps.append(m)
    return in_maps


# ------------------------------------------------------------------ driver
def kernel(x, edge_index, W1, as1, ad1, b1, W2, as2, ad2, b2,
           W3, as3, ad3, b3, Wm, bm, Wv, bv):
    global LAST_RESULT
    import os
    from concourse.bass_utils import run_bass_kernel_spmd

    T, off8, offT, wrap_src, dstloc16, dlocrep8 = _preprocess(
        np.asarray(edge_index))
    params = dict(W1=W1, as1=as1, ad1=ad1, b1=b1, W2=W2, as2=as2, ad2=ad2,
                  b2=b2, W3=W3, as3=as3, ad3=ad3, b3=b3, Wm=Wm, bm=bm,
                  Wv=Wv, bv=bv)
    in_maps = _make_in_maps(x, params, wrap_src, dstloc16, dlocrep8,
                            int(T.max()))

    nc = _build(T, off8, offT)
    res = run_bass_kernel_spmd(
        nc, in_maps, core_ids=list(range(NCORES)),
        trace=os.environ.get("BASS_TRACE", "") not in ("", "0"))
    LAST_RESULT = res

    z = np.concatenate([res.results[c]["z"] for c in range(NCORES)], axis=0)
    zm = np.concatenate([res.results[c]["zmean"] for c in range(NCORES)],
                        axis=0)
    zv = np.concatenate([res.results[c]["zvar"] for c in range(NCORES)],
                        axis=0)
    return zm, zv, z


# revision 21
# speedup vs baseline: 1.3960x; 1.3960x over previous
"""Distributed 3-layer GAT encoder on 8 TRN2 NeuronCores (Bass/Tile).

v2 strategy (graph partition by dst):
  - Core c owns dst nodes [2500c, 2500c+2500), padded to 2560 = 20 blocks x 128.
  - Self-loops are NOT in the edge list; their softmax contribution is folded
    into the flush using hloc_sb [P, 20, 264] (local rows [h|as|ad], SBUF
    resident, written by one matmul per block at the previous layer's flush).
  - Per layer, a full node table lives in each core's HBM:
      tab_l [20480, 384|128] fp16 : rows [h | alpha_src | alpha_dst | pad]
    built by matmuls from all-gathered transposed features with folded
    weights [W | W.a_src | W.a_dst].
  - Edge phase: ONE dma_gather per group of 4 dst blocks (the gather has a
    ~18us fixed cost, so fewer+bigger gathers win). Per 128-dst block:
      ind[e,d] by DVE compare of SBUF-resident dloc vs iota (no HBM stream);
      indT from an int8 HBM stream (dlocrep8) vs int8 iota;
      alpha_dst expanded per edge via matmul(lhsT=indT, rhs=hloc ad cols);
      p = exp(leaky_relu(as+ad)) computed on H cols only; the C-broadcast is
      folded into the DVE multiply p*h;
      numerator + denominator accumulated in PSUM via matmuls (lhsT=ind).
  - Flush: add self-loop terms, normalize, mean over heads, bias, relu ->
    PE transpose -> next-layer hloc matmul -> AllGather fp16 -> table rebuild.
"""
import numpy as np

N = 20000
NCORES = 8
NPC = 2500
NPAD = 2560
NBLK = 20
NTOT = NCORES * NPAD  # 20480
P = 128
AGBLK = 12  # blocks covered by the first AllGather chunk

LAST_RESULT = None


# ----------------------------------------------------------------- host prep
def _wrap16(idx, ncols):
    n = len(idx)
    w = np.zeros((P, ncols), dtype=np.int16)
    cols = (n + 15) // 16
    assert cols <= ncols
    buf = np.zeros((16, cols), dtype=np.int16)
    buf[np.arange(n) % 16, np.arange(n) // 16] = idx
    for g in range(8):
        w[16 * g:16 * g + 16, :cols] = buf
    return w


def _preprocess(edge_index):
    src = np.asarray(edge_index[0], dtype=np.int64)
    dst = np.asarray(edge_index[1], dtype=np.int64)
    # self-loops handled locally in the flush; NOT added to the edge list

    own_s = src // NPC
    src_p = own_s * NPAD + (src - own_s * NPC)
    own = dst // NPC
    dst_loc = dst - own * NPC

    order = np.lexsort((dst_loc, own))
    src_p, dst_loc, own = src_p[order], dst_loc[order], own[order]
    blk = dst_loc // P
    counts = np.zeros((NCORES, NBLK), dtype=np.int64)
    for c in range(NCORES):
        for b in range(NBLK):
            counts[c, b] = np.sum((own == c) & (blk == b))
    T = np.maximum(1, np.ceil(counts.max(axis=0) / P).astype(np.int64))
    Ttot = int(T.sum())

    wrap_src = np.zeros((NCORES, P, Ttot * 8), dtype=np.int16)
    dstloc16 = np.full((NCORES, P, Ttot), -1.0, dtype=np.float16)
    dlocrep8 = np.full((NCORES, Ttot * P), -1, dtype=np.int8)
    off8 = np.zeros(NBLK + 1, dtype=np.int64)
    offT = np.zeros(NBLK + 1, dtype=np.int64)
    for b in range(NBLK):
        off8[b + 1] = off8[b] + T[b] * 8
        offT[b + 1] = offT[b] + T[b]
    for c in range(NCORES):
        m_c = own == c
        for b in range(NBLK):
            m = m_c & (blk == b)
            cnt = int(counts[c, b])
            nb = int(T[b]) * P
            isrc = np.zeros(nb, dtype=np.int64)
            isrc[:cnt] = src_p[m]
            dl = np.full(nb, -1.0, dtype=np.float32)
            dl[:cnt] = dst_loc[m] - b * P
            wrap_src[c, :, off8[b]:off8[b + 1]] = _wrap16(isrc, int(T[b]) * 8)
            dstloc16[c, :, offT[b]:offT[b + 1]] = (
                dl.reshape(int(T[b]), P).T.astype(np.float16))
            dlocrep8[c, offT[b] * P:offT[b + 1] * P] = dl.astype(np.int8)
    # replicate along partitions: [NCORES, P, Ttot*P]
    dlocrep8 = np.repeat(dlocrep8[:, None, :], P, axis=1)
    return T, off8, offT, wrap_src, dstloc16, dlocrep8


# ------------------------------------------------------------- build program
def _build(T, off8, offT, do_compile=True):
    from concourse import bass, bacc, mybir, tile

    f16 = mybir.dt.float16
    f32 = mybir.dt.float32
    i16 = mybir.dt.int16
    i8 = mybir.dt.int8
    AF = mybir.ActivationFunctionType
    OP = mybir.AluOpType

    Ttot = int(T.sum())
    Tmax = int(T.max())
    NW = Ttot * 8
    NVALID_LAST = NPC - (NBLK - 1) * P  # 68

    nc = bacc.Bacc("TRN2", target_bir_lowering=False, debug=False,
                   num_devices=NCORES)

    # inputs
    xT16 = nc.dram_tensor("xT16", [P, NTOT], f16, kind="ExternalInput")
    xlocT = nc.dram_tensor("xlocT", [P, NPAD], f16, kind="ExternalInput")
    iwsrc = nc.dram_tensor("iwsrc", [P, NW], i16, kind="ExternalInput")
    dloc = nc.dram_tensor("dloc", [P, Ttot], f16, kind="ExternalInput")
    dlocrep8 = nc.dram_tensor("dlocrep8", [P, Ttot * P], i8,
                              kind="ExternalInput")
    iotabig = nc.dram_tensor("iotabig", [P, Tmax * P], f16,
                             kind="ExternalInput")
    iotacrbig8 = nc.dram_tensor("iotacrbig8", [P, Tmax * P], i8,
                                kind="ExternalInput")
    c100 = nc.dram_tensor("c100", [P, 32], f32, kind="ExternalInput")
    c1em8 = nc.dram_tensor("c1em8", [P, 32], f32, kind="ExternalInput")
    ident16 = nc.dram_tensor("ident16", [P, P], f16, kind="ExternalInput")
    identf = nc.dram_tensor("identf", [P, P], f32, kind="ExternalInput")
    # folded weights: [W | W.a_src | W.a_dst]
    w1c = nc.dram_tensor("w1c", [128, 264], f16, kind="ExternalInput")
    w2c = nc.dram_tensor("w2c", [64, 264], f16, kind="ExternalInput")
    w3c = nc.dram_tensor("w3c", [64, 34], f16, kind="ExternalInput")
    b1r = nc.dram_tensor("b1r", [P, 64], f32, kind="ExternalInput")
    b2r = nc.dram_tensor("b2r", [P, 64], f32, kind="ExternalInput")
    b3r = nc.dram_tensor("b3r", [P, 32], f32, kind="ExternalInput")
    bmr = nc.dram_tensor("bmr", [P, 32], f32, kind="ExternalInput")
    bvr = nc.dram_tensor("bvr", [P, 32], f32, kind="ExternalInput")
    wm = nc.dram_tensor("wm", [32, 32], f32, kind="ExternalInput")
    wv = nc.dram_tensor("wv", [32, 32], f32, kind="ExternalInput")

    # outputs
    z_out = nc.dram_tensor("z", [NPC, 32], f32, kind="ExternalOutput")
    zm_out = nc.dram_tensor("zmean", [NPC, 32], f32, kind="ExternalOutput")
    zv_out = nc.dram_tensor("zvar", [NPC, 32], f32, kind="ExternalOutput")

    with tile.TileContext(nc) as tc:
        with (
            tc.tile_pool(name="const", bufs=1) as cpool,
            tc.tile_pool(name="sb", bufs=3) as sb,
            tc.tile_pool(name="gth", bufs=6) as gth,
            tc.tile_pool(name="blk", bufs=3) as blk,
            tc.tile_pool(name="blks", bufs=3) as blks,
            tc.tile_pool(name="psreb", bufs=2, space="PSUM") as psreb,
            tc.tile_pool(name="psad", bufs=1, space="PSUM") as psad,
            tc.tile_pool(name="pssm", bufs=1, space="PSUM") as pssm,
            tc.tile_pool(name="psagg", bufs=2, space="PSUM") as psagg,
            tc.tile_pool(name="dram", bufs=1, space="DRAM") as dram,
        ):
            tab1 = dram.tile([NTOT, 384], f16)
            tab2 = dram.tile([NTOT, 384], f16)
            tab3 = dram.tile([NTOT, 128], f16)
            x2T_locA = dram.tile([64, AGBLK * P], f16)
            x2T_locB = dram.tile([64, NPAD - AGBLK * P], f16)
            x3T_locA = dram.tile([64, AGBLK * P], f16)
            x3T_locB = dram.tile([64, NPAD - AGBLK * P], f16)
            CAG = AGBLK * P  # column split for the chunked AllGather
            x2T_fullA = dram.tile([NCORES, 64, CAG], f16)
            x2T_fullB = dram.tile([NCORES, 64, NPAD - CAG], f16)
            x3T_fullA = dram.tile([NCORES, 64, CAG], f16)
            x3T_fullB = dram.tile([NCORES, 64, NPAD - CAG], f16)

            def ld(shape, dt, src):
                t = cpool.tile(shape, dt, tag="c_" + src.name)
                nc.sync.dma_start(out=t[:], in_=src[:, :])
                return t

            id16_sb = ld([P, P], f16, ident16)
            idf_sb = ld([P, P], f32, identf)
            w1c_sb = ld([128, 264], f16, w1c)
            w2c_sb = ld([64, 264], f16, w2c)
            w3c_sb = ld([64, 34], f16, w3c)
            b1r_sb = ld([P, 64], f32, b1r)
            b2r_sb = ld([P, 64], f32, b2r)
            b3r_sb = ld([P, 32], f32, b3r)
            bmr_sb = ld([P, 32], f32, bmr)
            bvr_sb = ld([P, 32], f32, bvr)
            wm_sb = ld([32, 32], f32, wm)
            wv_sb = ld([32, 32], f32, wv)
            iwsrc_sb = ld([P, NW], i16, iwsrc)
            dloc_sb = ld([P, Ttot], f16, dloc)
            iotabig_sb = ld([P, Tmax * P], f16, iotabig)
            iotacrbig8_sb = ld([P, Tmax * P], i8, iotacrbig8)
            c100_sb = ld([P, 32], f32, c100)
            c1em8_sb = ld([P, 32], f32, c1em8)
            xloc_sb = ld([P, NPAD], f16, xlocT)

            # local rows [h | as | ad] of the CURRENT layer, per dst block
            hloc_sb = cpool.tile([P, NBLK, 264], f16, tag="hloc")

            # -------- table rebuild: tab rows = fp16(xT^T @ wc) ----------
            G = 4
            NGR = NBLK // G  # 5 groups of 4 blocks per core
            def rebuild(src_getter, wc_sb, in_c, ncols, tab, groups):
                for it, (c, g) in enumerate(
                        (c, g) for c in range(NCORES) for g in groups):
                    e1 = nc.sync if it % 2 == 0 else nc.scalar
                    e2 = nc.scalar if it % 2 == 0 else nc.sync
                    lh = sb.tile([in_c, G * P], f16, tag="reblh")
                    e1.dma_start(out=lh[:], in_=src_getter(c, g))
                    h16 = sb.tile([P, G, ncols], f16, tag="rebh")
                    for j in range(G):
                        pr = psreb.tile([P, ncols], f32, space="PSUM",
                                        tag="reb")
                        nc.tensor.matmul(
                            out=pr[:], lhsT=lh[:, j * P:(j + 1) * P],
                            rhs=wc_sb[:in_c, :ncols], start=True, stop=True)
                        if j % 2 == 0:
                            nc.vector.tensor_copy(out=h16[:, j, :], in_=pr[:])
                        else:
                            nc.scalar.activation(h16[:, j, :], pr[:], AF.Copy)
                    r0 = c * NPAD + g * G * P
                    e2.dma_start(
                        out=tab[r0:r0 + G * P, 0:ncols]
                        .rearrange("(j r) c -> r j c", j=G),
                        in_=h16[:])

            # -------- hloc for layer 1 (from local transposed x) ---------
            def hloc_init():
                for b in range(NBLK):
                    pr = pssm.tile([P, 264], f32, space="PSUM", tag="hl")
                    nc.tensor.matmul(out=pr[:],
                                     lhsT=xloc_sb[:, b * P:(b + 1) * P],
                                     rhs=w1c_sb[:, :], start=True, stop=True)
                    if b % 2 == 0:
                        nc.vector.tensor_copy(out=hloc_sb[:, b, :], in_=pr[:])
                    else:
                        nc.scalar.activation(hloc_sb[:, b, :], pr[:], AF.Copy)

            # -------- edge phase ------------------------------------------
            def edge_layer(tab, elem, H, C, flush, post_flush=None):
                HC = H * C
                for b in range(NBLK):
                    Tb = int(T[b])
                    nidx = Tb * P
                    g = gth.tile([P, Tb, elem], f16, tag="g")
                    nc.gpsimd.dma_gather(
                        out_ap=g[:], in_ap=tab[:, :],
                        idxs_ap=iwsrc_sb[:, int(off8[b]):int(off8[b + 1])],
                        num_idxs=nidx, num_idxs_reg=nidx, elem_size=elem,
                        elem_step=int(tab.shape[1]),
                        single_packet=nidx <= 1024)
                    if True:
                        gb = g[:, :, :]
                        dlr8 = blks.tile([P, Tb * P], i8, tag="dlr8")
                        nc.sync.dma_start(
                            out=dlr8[:],
                            in_=dlocrep8[:, int(offT[b]) * P:
                                         int(offT[b + 1]) * P])
                        ind = blks.tile([P, Tb, P], f16, tag="ind")
                        nc.vector.tensor_tensor(
                            out=ind[:],
                            in0=dloc_sb[:, int(offT[b]):int(offT[b + 1]),
                                        None].to_broadcast([P, Tb, P]),
                            in1=iotabig_sb[:, :Tb * P]
                            .rearrange("p (t q) -> p t q", t=Tb),
                            op=OP.is_equal)
                        indT = blks.tile([P, Tb, P], f16, tag="indT")
                        nc.vector.tensor_tensor(
                            out=indT[:].rearrange("p t q -> p (t q)"),
                            in0=iotacrbig8_sb[:, :Tb * P],
                            in1=dlr8[:], op=OP.is_equal)
                        pad_all = psad.tile([P, Tb, H], f32, space="PSUM",
                                            tag="ad")
                        for t in range(Tb):
                            nc.tensor.matmul(
                                out=pad_all[:, t, :],
                                lhsT=indT[:, t, :],
                                rhs=hloc_sb[:, b, HC + H:HC + 2 * H],
                                start=True, stop=True)
                        es = sb.tile([P, Tb, H], f32, tag="es")
                        nc.vector.tensor_add(out=es[:],
                                             in0=gb[:, :, HC:HC + H],
                                             in1=pad_all[:])
                        es2 = sb.tile([P, Tb, H], f32, tag="es2")
                        nc.vector.tensor_scalar_mul(out=es2[:], in0=es[:],
                                                    scalar1=0.2)
                        nc.vector.tensor_max(out=es[:], in0=es[:], in1=es2[:])
                        pex = blk.tile([P, Tb, HC + H], f16, tag="pex")
                        nc.scalar.activation(
                            pex[:, :, 0:HC]
                            .rearrange("p t (h c) -> p t h c", h=H),
                            es[:, :, :, None].to_broadcast([P, Tb, H, C]),
                            AF.Exp)
                        nc.scalar.activation(pex[:, :, HC:HC + H], es[:],
                                             AF.Exp)
                        nc.vector.tensor_mul(out=pex[:, :, 0:HC],
                                             in0=gb[:, :, 0:HC],
                                             in1=pex[:, :, 0:HC])
                        pa = psagg.tile([P, HC + H], f32, space="PSUM",
                                        tag="agg")
                        for t in range(Tb):
                            nc.tensor.matmul(
                                out=pa[:], lhsT=ind[:, t, :],
                                rhs=pex[:, t, :],
                                start=(t == 0), stop=(t == Tb - 1))
                        flush(b, pa)
                        if post_flush is not None:
                            post_flush(b)

            # -------- self-loop contribution (p_self, numer, denom) ------
            def self_terms(b, pa, H, C):
                HC = H * C
                est = sb.tile([P, H], f32, tag="est")
                nc.vector.tensor_add(out=est[:],
                                     in0=hloc_sb[:, b, HC:HC + H],
                                     in1=hloc_sb[:, b, HC + H:HC + 2 * H])
                es2t = sb.tile([P, H], f32, tag="es2t")
                nc.vector.tensor_scalar_mul(out=es2t[:], in0=est[:],
                                            scalar1=0.2)
                nc.vector.tensor_max(out=est[:], in0=est[:], in1=es2t[:])
                psf = sb.tile([P, H], f32, tag="psf")
                nc.scalar.activation(psf[:], est[:], AF.Exp)
                den = sb.tile([P, H], f32, tag="den")
                nc.vector.tensor_add(out=den[:], in0=pa[:, HC:HC + H],
                                     in1=psf[:])
                num = sb.tile([P, HC], f32, tag="num")
                nc.vector.tensor_tensor(
                    out=num[:].rearrange("p (h c) -> p h c", h=H),
                    in0=hloc_sb[:, b, 0:HC]
                    .rearrange("p (h c) -> p h c", h=H),
                    in1=psf[:, :, None].to_broadcast([P, H, C]),
                    op=OP.mult)
                nc.vector.tensor_add(out=num[:], in0=num[:], in1=pa[:, 0:HC])
                return num, den

            # -------- flush -----------------------------------------------
            def flush_12(b, pa, H, C, brep_sb, xT_locAB, wnext_sb, wn_cols):
                HC = H * C
                num, den = self_terms(b, pa, H, C)
                inv = sb.tile([P, H], f32, tag="inv")
                nc.vector.tensor_scalar_add(out=inv[:], in0=den[:],
                                            scalar1=1e-16)
                nc.vector.reciprocal(out=inv[:], in_=inv[:])
                nc.vector.tensor_scalar_mul(out=inv[:], in0=inv[:],
                                            scalar1=1.0 / H)
                ivx = sb.tile([P, HC], f32, tag="ivx")
                nc.scalar.activation(
                    ivx[:].rearrange("p (h c) -> p h c", h=H),
                    inv[:, :, None].to_broadcast([P, H, C]), AF.Copy)
                nrm = sb.tile([P, HC], f32, tag="nrm")
                nc.vector.tensor_mul(out=nrm[:], in0=num[:], in1=ivx[:])
                m = sb.tile([P, C], f32, tag="mean")
                nc.vector.tensor_reduce(
                    out=m[:], in_=nrm[:].rearrange("p (h c) -> p c h", h=H),
                    axis=mybir.AxisListType.X, op=OP.add)
                nc.vector.tensor_add(out=m[:], in0=m[:], in1=brep_sb[:, :C])
                x16 = sb.tile([P, C], f16, tag="x16")
                nc.scalar.activation(x16[:], m[:], AF.Relu)
                pt = pssm.tile([C, P], f16, space="PSUM", tag="sm")
                nc.tensor.transpose(out=pt[:], in_=x16[:], identity=id16_sb[:])
                xt = sb.tile([C, P], f16, tag="xt")
                nc.scalar.activation(xt[:], pt[:], AF.Copy)
                if b < AGBLK:
                    nc.sync.dma_start(
                        out=xT_locAB[0][:, b * P:(b + 1) * P], in_=xt[:])
                else:
                    c0 = (b - AGBLK) * P
                    nc.sync.dma_start(
                        out=xT_locAB[1][:, c0:c0 + P], in_=xt[:])
                # next layer's local rows [h | as | ad]
                prh = pssm.tile([P, wn_cols], f32, space="PSUM", tag="hl")
                nc.tensor.matmul(out=prh[:], lhsT=xt[:],
                                 rhs=wnext_sb[:C, :wn_cols],
                                 start=True, stop=True)
                nc.vector.tensor_copy(out=hloc_sb[:, b, 0:wn_cols],
                                      in_=prh[:])

            def flush_3(b, pa):
                nvalid = NVALID_LAST if b == NBLK - 1 else P
                num, den = self_terms(b, pa, 1, 32)
                inv = sb.tile([P, 1], f32, tag="inv")
                nc.vector.tensor_scalar_add(out=inv[:], in0=den[:],
                                            scalar1=1e-16)
                nc.vector.reciprocal(out=inv[:], in_=inv[:])
                z = sb.tile([P, 32], f32, tag="zf")
                nc.vector.tensor_scalar_mul(out=z[:], in0=num[:],
                                            scalar1=inv[:])
                nc.vector.tensor_add(out=z[:], in0=z[:], in1=b3r_sb[:])
                nc.sync.dma_start(out=z_out[b * P:b * P + nvalid, :],
                                  in_=z[:nvalid, :])
                zt_ps = pssm.tile([32, P], f32, space="PSUM", tag="sm")
                nc.tensor.transpose(out=zt_ps[:], in_=z[:, :32],
                                    identity=idf_sb[:])
                zt = sb.tile([32, P], f32, tag="zt")
                nc.vector.tensor_copy(out=zt[:], in_=zt_ps[:])
                pm = pssm.tile([P, 32], f32, space="PSUM", tag="sm2")
                nc.tensor.matmul(out=pm[:], lhsT=zt[:], rhs=wm_sb[:],
                                 start=True, stop=True)
                zm = sb.tile([P, 32], f32, tag="zm")
                nc.vector.tensor_add(out=zm[:], in0=pm[:], in1=bmr_sb[:])
                nc.sync.dma_start(out=zm_out[b * P:b * P + nvalid, :],
                                  in_=zm[:nvalid, :])
                pv = pssm.tile([P, 32], f32, space="PSUM", tag="sm2")
                nc.tensor.matmul(out=pv[:], lhsT=zt[:], rhs=wv_sb[:],
                                 start=True, stop=True)
                zv = sb.tile([P, 32], f32, tag="zv")
                nc.vector.tensor_add(out=zv[:], in0=pv[:], in1=bvr_sb[:])
                nc.scalar.activation(zv[:], zv[:], AF.Exp)
                nc.vector.tensor_tensor(out=zv[:], in0=zv[:], in1=c100_sb[:],
                                        op=OP.min)
                nc.vector.tensor_tensor(out=zv[:], in0=zv[:], in1=c1em8_sb[:],
                                        op=OP.max)
                nc.sync.dma_start(out=zv_out[b * P:b * P + nvalid, :],
                                  in_=zv[:nvalid, :])

            # ================ the program ==================================
            def ag_chunks(locA, locB, fullA, fullB):
                def post(b):
                    if b == AGBLK - 1:
                        nc.gpsimd.collective_compute(
                            "AllGather", mybir.AluOpType.bypass,
                            replica_groups=[list(range(NCORES))],
                            ins=[locA[:]], outs=[fullA[:]])
                    elif b == NBLK - 1:
                        nc.gpsimd.collective_compute(
                            "AllGather", mybir.AluOpType.bypass,
                            replica_groups=[list(range(NCORES))],
                            ins=[locB[:]], outs=[fullB[:]])
                return post

            GRP_A = range(AGBLK // G)        # block groups 0..2
            GRP_B = range(AGBLK // G, NGR)   # block groups 3..4

            def srcAB(fullA, fullB):
                def get(c, g):
                    c0 = g * G * P
                    if c0 < CAG:
                        return fullA[c, :, c0:c0 + G * P]
                    return fullB[c, :, c0 - CAG:c0 - CAG + G * P]
                return get

            hloc_init()
            rebuild(lambda c, g: xT16[:, c * NPAD + g * G * P:
                                      c * NPAD + (g + 1) * G * P],
                    w1c_sb, 128, 264, tab1, range(NGR))
            edge_layer(tab1, 384, 4, 64,
                       lambda b, pa: flush_12(b, pa, 4, 64, b1r_sb,
                                              (x2T_locA, x2T_locB),
                                              w2c_sb, 264),
                       post_flush=ag_chunks(x2T_locA, x2T_locB,
                                            x2T_fullA, x2T_fullB))
            rebuild(srcAB(x2T_fullA, x2T_fullB), w2c_sb, 64, 264, tab2,
                    GRP_A)
            rebuild(srcAB(x2T_fullA, x2T_fullB), w2c_sb, 64, 264, tab2,
                    GRP_B)
            edge_layer(tab2, 384, 4, 64,
                       lambda b, pa: flush_12(b, pa, 4, 64, b2r_sb,
                                              (x3T_locA, x3T_locB),
                                              w3c_sb, 34),
                       post_flush=ag_chunks(x3T_locA, x3T_locB,
                                            x3T_fullA, x3T_fullB))
            rebuild(srcAB(x3T_fullA, x3T_fullB), w3c_sb, 64, 34, tab3,
                    GRP_A)
            rebuild(srcAB(x3T_fullA, x3T_fullB), w3c_sb, 64, 34, tab3,
                    GRP_B)
            edge_layer(tab3, 128, 1, 32, flush_3)

    if do_compile:
        nc.compile()
    return nc


def _make_in_maps(x, params, wrap_src, dstloc16, dlocrep8, Tmax):
    x = np.asarray(x, dtype=np.float32)

    def comb(W, a_s, a_d):
        W = np.asarray(W, np.float32)
        a_s = np.asarray(a_s, np.float32)
        a_d = np.asarray(a_d, np.float32)
        heads, c = a_s.shape
        Wr = W.reshape(W.shape[0], heads, c)
        was = np.einsum('ihc,hc->ih', Wr, a_s)
        wad = np.einsum('ihc,hc->ih', Wr, a_d)
        return np.concatenate([W, was, wad], axis=1).astype(np.float16)

    xT16 = np.zeros((P, NTOT), dtype=np.float16)
    for c in range(NCORES):
        xs = x[c * NPC:(c + 1) * NPC]
        xT16[:, c * NPAD:c * NPAD + NPC] = xs.T.astype(np.float16)

    def rep(v, n=P):
        v = np.asarray(v, np.float32).reshape(1, -1)
        return np.repeat(v, n, axis=0).astype(np.float32)

    iotacr8 = np.tile(np.arange(P, dtype=np.int8).reshape(P, 1),
                      (1, Tmax * P))
    common = dict(
        xT16=xT16,
        iotabig=np.tile(np.arange(P, dtype=np.float16), (P, Tmax)),
        iotacrbig8=iotacr8,
        c100=np.full((P, 32), 100.0, dtype=np.float32),
        c1em8=np.full((P, 32), 1e-8, dtype=np.float32),
        ident16=np.eye(P, dtype=np.float16),
        identf=np.eye(P, dtype=np.float32),
        w1c=comb(params['W1'], params['as1'], params['ad1']),
        w2c=comb(params['W2'], params['as2'], params['ad2']),
        w3c=comb(params['W3'], params['as3'], params['ad3']),
        b1r=rep(params['b1']), b2r=rep(params['b2']), b3r=rep(params['b3']),
        bmr=rep(params['bm']), bvr=rep(params['bv']),
        wm=np.asarray(params['Wm'], np.float32),
        wv=np.asarray(params['Wv'], np.float32),
    )
    in_maps = []
    for c in range(NCORES):
        m = dict(common)
        m.update(iwsrc=wrap_src[c], dloc=dstloc16[c], dlocrep8=dlocrep8[c],
                 xlocT=xT16[:, c * NPAD:(c + 1) * NPAD].copy())
        in_maps.append(m)
    return in_maps


# ------------------------------------------------------------------ driver
def kernel(x, edge_index, W1, as1, ad1, b1, W2, as2, ad2, b2,
           W3, as3, ad3, b3, Wm, bm, Wv, bv):
    global LAST_RESULT
    import os
    from concourse.bass_utils import run_bass_kernel_spmd

    T, off8, offT, wrap_src, dstloc16, dlocrep8 = _preprocess(
        np.asarray(edge_index))
    params = dict(W1=W1, as1=as1, ad1=ad1, b1=b1, W2=W2, as2=as2, ad2=ad2,
                  b2=b2, W3=W3, as3=as3, ad3=ad3, b3=b3, Wm=Wm, bm=bm,
                  Wv=Wv, bv=bv)
    in_maps = _make_in_maps(x, params, wrap_src, dstloc16, dlocrep8,
                            int(T.max()))

    nc = _build(T, off8, offT)
    res = run_bass_kernel_spmd(
        nc, in_maps, core_ids=list(range(NCORES)),
        trace=os.environ.get("BASS_TRACE", "") not in ("", "0"))
    LAST_RESULT = res

    z = np.concatenate([res.results[c]["z"] for c in range(NCORES)], axis=0)
    zm = np.concatenate([res.results[c]["zmean"] for c in range(NCORES)],
                        axis=0)
    zv = np.concatenate([res.results[c]["zvar"] for c in range(NCORES)],
                        axis=0)
    return zm, zv, z
